# revision 32
# baseline (speedup 1.0000x reference)
"""Multi-head causal self-attention (B=1, S=4096, D=1024, H=16) on 8 TRN2 cores.

Sharding: 2 heads per core (head/tensor parallel). Each core computes its
heads' Q/K/V projections, causal flash attention, and a partial output
projection against its 128 columns of Wo. The host sums the 8 partials and
adds the output bias.

Device layouts (per core, bf16 compute):
  - x is fed transposed:  xT [D=1024, S=4096]   (model dim on partitions)
  - Q^T, K^T [128, 4096]: per-core head dims on partitions (h0: 0-63, h1: 64-127)
  - V natural [4096, 130]: per seq-tile [128, 65*2] = [V_h0 | ones | V_h1 | ones]
    The ones column makes the PV matmul also produce the softmax denominator.
  - scores are computed transposed S^T[k, q] so the PV matmul needs no
    transposition; softmax is exp-only (scores are bounded, no max-subtract).
  - the per-query 1/denominator is broadcast across the 64 head-feature
    partitions with K=1 ones-vector PE matmuls (h1 at tile-position column
    64), then per-head multiplies write normalized attnT rows directly.
  - output is written transposed outT [1024, 4096] fp16 (partial; host sums).
"""

import numpy as np
import ml_dtypes
from contextlib import ExitStack

import concourse.bass as bass
import concourse.tile as tile
from concourse import bacc, mybir
from concourse.bass_utils import run_bass_kernel_spmd

P = 128
S = 4096
D = 1024
DH = 64
N_CORES = 8
SCALE = 1.0 / 8.0  # 1/sqrt(64)
NQ = 512           # query block (matmul free dim)
KT = 128           # key tile (contraction partitions)
NQB = S // NQ      # 8 query blocks
NKT = S // KT      # 32 key tiles
KO = D // P        # 8 contraction subtiles over the model dim

BF16 = mybir.dt.bfloat16
F16 = mybir.dt.float16
F32 = mybir.dt.float32
EXP = mybir.ActivationFunctionType.Exp
ADD = mybir.AluOpType.add


def _emit(tc, xT, wqT, wkT, wvT, woT, bqk, masks, outT, dbg=None):
    nc = tc.nc
    with ExitStack() as ctx:
        from collections import deque
        from concourse.masks import make_identity

        const = ctx.enter_context(tc.tile_pool(name="const", bufs=1))

        # DMA order follows first-use: wq + x chunk-0-lo unblock the first
        # projection matmuls; don't queue anything behind the 8MB of xT.
        ones64 = const.tile([1, DH], BF16)
        nc.vector.memset(ones64, 1.0)

        xT_r = xT.rearrange("(ko p) n -> p ko n", p=P)
        xT_sb = const.tile([P, KO, S], BF16)
        wq_sb = const.tile([P, KO, P], BF16)
        nc.sync.dma_start(wq_sb, wqT.rearrange("(ko p) m -> p ko m", p=P))
        nc.sync.dma_start(xT_sb[:, 0:4, 0:NQ], xT_r[:, 0:4, 0:NQ])
        wk_sb = const.tile([P, KO, P], BF16)
        nc.sync.dma_start(wk_sb, wkT.rearrange("(ko p) m -> p ko m", p=P))
        nc.sync.dma_start(xT_sb[:, 4:8, 0:NQ], xT_r[:, 4:8, 0:NQ])
        wv_sb = const.tile([P, KO, P], BF16)
        nc.sync.dma_start(wv_sb, wvT.rearrange("(ko p) m -> p ko m", p=P))
        bqk_sb = const.tile([P, 3], F32)
        nc.sync.dma_start(bqk_sb, bqk.rearrange("b p -> p b"))
        masks_sb = const.tile([P, 4, NQ], BF16)
        nc.sync.dma_start(masks_sb, masks)
        nc.sync.dma_start(xT_sb[:, :, NQ:2 * NQ], xT_r[:, :, NQ:2 * NQ])
        wo_sb = const.tile([P, D], BF16)
        nc.sync.dma_start(wo_sb, woT)
        for n in range(2, NQB):  # chunked so projections can start early
            nsl = slice(n * NQ, (n + 1) * NQ)
            nc.sync.dma_start(xT_sb[:, :, nsl], xT_r[:, :, nsl])

        qT_sb = const.tile([P, S], BF16)
        kT_sb = const.tile([P, S], BF16)
        vT_sb = const.tile([P, S], BF16)
        v_sb = const.tile([P, S // P, 130], BF16)
        attnT_sb = const.tile([P, S], BF16)
        nc.gpsimd.memset(v_sb, 1.0)  # presets the two ones-columns

        ident = const.tile([P, P], BF16)
        make_identity(nc, ident)

        # Warm the PE clock (HAM) with throwaway matmuls while the input DMAs
        # land: ~20 matmuls of N=128 ~ 3us of sustained PE activity.
        with tc.tile_pool(name="warm_psum", bufs=1, space="PSUM") as wpool:
            wt = wpool.tile([P, P], F32)
            for _ in range(32):
                nc.tensor.matmul(wt, lhsT=ident, rhs=ident, start=True, stop=True)

        # PSUM budget (8 banks): spool 4 (two [128,2,512] score slabs),
        # vpool 2 (pv0/pv1 accumulators), ppool 1 (proj accum / V transpose),
        # opool 1 (output projection / reciprocal broadcast).
        spool = ctx.enter_context(tc.tile_pool(name="score_psum", bufs=2, space="PSUM"))
        vpool = ctx.enter_context(tc.tile_pool(name="pv_psum", bufs=1, space="PSUM"))
        ppool = ctx.enter_context(tc.tile_pool(name="proj_psum", bufs=1, space="PSUM"))
        opool = ctx.enter_context(tc.tile_pool(name="oproj_psum", bufs=1, space="PSUM"))
        work = ctx.enter_context(tc.tile_pool(name="work", bufs=5))
        nwork = ctx.enter_context(tc.tile_pool(name="nwork", bufs=3))

        def proj_chunk(bcol, w_sb, dst, n):
            """Two pacing items of 4 accumulation matmuls each (shared psum)."""
            state = {}

            def emit_lo():
                ps = ppool.tile([P, NQ], F32, tag="ps", name=f"ps_{bcol}_{n}")
                state["ps"] = ps
                for kt in range(KO // 2):
                    nc.tensor.matmul(
                        ps,
                        lhsT=w_sb[:, kt, :],
                        rhs=xT_sb[:, kt, n * NQ:(n + 1) * NQ],
                        start=(kt == 0),
                        stop=False,
                    )

            def emit_hi():
                ps = state["ps"]
                for kt in range(KO // 2, KO):
                    nc.tensor.matmul(
                        ps,
                        lhsT=w_sb[:, kt, :],
                        rhs=xT_sb[:, kt, n * NQ:(n + 1) * NQ],
                        start=False,
                        stop=(kt == KO - 1),
                    )
                nc.vector.tensor_tensor(
                    dst[:, n * NQ:(n + 1) * NQ],
                    ps,
                    bqk_sb[:, bcol:bcol + 1].to_broadcast([P, NQ]),
                    op=ADD,
                )

            return [emit_lo, emit_hi]

        def v_transpose(t):
            def emit():
                tp = ppool.tile([P, P], BF16, tag="ps", name=f"tp_{t}")
                nc.tensor.transpose(tp, vT_sb[:, t * P:(t + 1) * P], ident)
                nc.vector.tensor_copy(
                    v_sb[:, t, :].rearrange("p (h x) -> p h x", x=65)[:, :, 0:DH],
                    tp.rearrange("p (h x) -> p h x", x=DH),
                )
            return emit

        def proj_ops(nb):
            ops = []
            ops += proj_chunk(0, wq_sb, qT_sb, nb)
            ops += proj_chunk(1, wk_sb, kT_sb, nb)
            ops += proj_chunk(2, wv_sb, vT_sb, nb)
            ops += [v_transpose(t) for t in range(4 * nb, 4 * nb + 4)]
            return ops

        def oproj_mtile(b, m, ptag="po", pool=None, gp=False, cs=None):
            def emit():
                c0, c1 = (cs.start, cs.stop) if cs else (0, NQ)
                n = c1 - c0
                qsl = slice(b * NQ + c0, b * NQ + c1)
                po = (pool or opool).tile(
                    [P, n], F32, tag=ptag, name=f"po_{b}_{m}_{c0}")
                nc.tensor.matmul(
                    po,
                    lhsT=wo_sb[:, m * P:(m + 1) * P],
                    rhs=attnT_sb[:, qsl],
                    start=True,
                    stop=True,
                )
                ot = work.tile([P, n], F16, tag="ot", name=f"ot_{b}_{m}_{c0}")
                if gp:
                    nc.scalar.copy(ot, po)
                else:
                    nc.vector.tensor_copy(ot, po)
                nc.sync.dma_start(
                    outT.rearrange("(mo p) n -> p mo n", p=P)[:, m, qsl], ot
                )
            return emit

        # block 0's projections up front
        for op in proj_ops(0):
            op()

        # Deferred output-projection queue: blocks 0-6 enqueue their 8
        # m-tiles after their attnT is normalized, but the work is only
        # popped during blocks >= 5, whose long kt loops are exp-bound -
        # the PE needs ready filler there to avoid head-of-line stalls on
        # PV-waiting-for-exp (each stall also drops the PE p-state).
        oproj_g = deque()

        def normalize_part1(b, pvs, cs=None, sfx=""):
            """End-of-block: drain PV psum to SBUF + reciprocal of the
            denominators. Frees the pv banks as early as possible. h1's
            rows are staged at partition base 64 (cross-partition DVE copy)
            so its normalize multiply can write attnT rows 64-127 directly."""
            cs = cs or slice(0, NQ)
            n = cs.stop - cs.start
            dcp = nwork.tile([1, 2, n], F32, tag="dcp", name=f"dcp_{b}{sfx}")
            pvS = nwork.tile([P, n], F32, tag="pvS", name=f"pvS_{b}{sfx}")
            for h in (0, 1):
                # denominator rows -> one base-0 tile (heads packed in the
                # free dim: custom-DVE recip mishandles nonzero partition
                # bases, and partition-base-1 writes are illegal)
                nc.vector.tensor_copy(dcp[:, h, :], pvs[h][DH:DH + 1, cs])
                nc.vector.tensor_copy(
                    pvS[h * DH:(h + 1) * DH, :], pvs[h][0:DH, cs])
            nc.vector.reciprocal_approx_fast(dcp, dcp)
            dcpb = nwork.tile([1, 2, n], BF16, tag="dcpb", name=f"dcpb_{b}{sfx}")
            nc.vector.tensor_copy(dcpb, dcp)
            return pvS, dcpb

        def normalize_part2(b, pvS, dcpb, cs=None, sfx=""):
            """Broadcast 1/denom over the head-feature partitions with K=1
            ones-vector matmuls (into the pv banks just freed by part1,
            h1's at tile-position column 64), then two multiplies write
            normalized attnT rows directly. Deferred into the next block's
            kt loop so the block boundary never stalls the PE queue."""
            def emit():
                cs_ = cs or slice(0, NQ)
                n = cs_.stop - cs_.start
                qsl = slice(b * NQ + cs_.start, b * NQ + cs_.stop)
                bbp = [
                    vpool.tile([P, n], F32, tag=f"pv{h}", name=f"bbp{h}_{b}{sfx}")
                    for h in (0, 1)
                ]
                for h in (0, 1):
                    nc.tensor.matmul(
                        bbp[h][h * DH:(h + 1) * DH, :],
                        lhsT=ones64,
                        rhs=dcpb[:, h, :],
                        start=True,
                        stop=True,
                    )
                    nc.vector.tensor_mul(
                        attnT_sb[h * DH:(h + 1) * DH, qsl],
                        pvS[h * DH:(h + 1) * DH, :],
                        bbp[h][h * DH:(h + 1) * DH, :],
                    )
                if b < NQB - 1:
                    oproj_g.extend(oproj_mtile(b, m) for m in range(D // P))
            return emit

        def pe_warm(n):
            # throwaway matmuls: keep the PE activity monitor dense so the
            # clock does not downshift while waiting on the exp stream
            dpo = opool.tile([P, P], F32, tag="po", name="dpo")
            for _ in range(n):
                nc.tensor.matmul(dpo, lhsT=ident, rhs=ident, start=True, stop=True)

        norm_q = deque()
        for b in range(NQB):
            nk = 4 * (b + 1)  # causal: only key tiles up to the diagonal
            proj_q = deque(proj_ops(b + 1)) if b + 1 < NQB else deque()
            pvs = [
                vpool.tile([DH + 1, NQ], F32, tag=f"pv{h}", name=f"pv{h}_{b}")
                for h in (0, 1)
            ]

            def emit_pv(st):
                pT, kt_, q0, nq, ki_ = st
                for h in (0, 1):
                    nc.tensor.matmul(
                        pvs[h][:, q0:],
                        lhsT=v_sb[:, kt_, h * 65:(h + 1) * 65],
                        rhs=pT[:, h, :nq],
                        start=(ki_ == 0),
                        stop=(ki_ == nk - 1),
                    )

            prev = None  # PV runs one k-tile behind the scores/exp pipeline
            for ki, kt in enumerate(range(nk)):
                j = kt - 4 * b  # >= 0 on causal-diagonal key tiles
                # on diagonal tiles only queries >= 128j can attend this tile
                q0 = max(0, j) * KT
                nq = NQ - q0
                qs0 = b * NQ + q0
                slab = spool.tile([P, 2, NQ], F32, tag="slab")
                for h in (0, 1):
                    nc.tensor.matmul(
                        slab[:, h, :nq],
                        lhsT=kT_sb[h * DH:(h + 1) * DH, kt * KT:(kt + 1) * KT],
                        rhs=qT_sb[h * DH:(h + 1) * DH, qs0:qs0 + nq],
                        start=True,
                        stop=True,
                    )
                pT = work.tile([P, 2, NQ], BF16, tag="pT")
                nc.scalar.activation(pT[:, :, :nq], slab[:, :, :nq], EXP, scale=SCALE)
                if j >= 0:
                    for h in (0, 1):
                        nc.vector.tensor_mul(
                            pT[:, h, :nq],
                            pT[:, h, :nq],
                            masks_sb[:, j, q0:],
                        )
                if prev is not None:
                    emit_pv(prev)
                prev = (pT, kt, q0, nq, ki)
                # PE filler priority: finish the previous block's normalize
                # (delayed 2 kt so its vector chain clears first), then next
                # block's projections, then deferred output projections
                # (concentrated in the exp-bound blocks 6-7), else throwaway
                # warm matmuls so the PE clock stays up while exp catches up
                if norm_q and ki >= 2:
                    norm_q.popleft()()
                elif proj_q:
                    proj_q.popleft()()
                elif oproj_g and b >= 5:
                    oproj_g.popleft()()
                    if b == NQB - 1 and ki >= 17 and oproj_g:
                        oproj_g.popleft()()
                elif b >= 4:
                    pe_warm(3)
            emit_pv(prev)
            while norm_q:
                norm_q.popleft()()
            while proj_q:
                proj_q.popleft()()
            pvS, dcpb = normalize_part1(b, pvs)
            norm_q.append(normalize_part2(b, pvS, dcpb))
        while norm_q:
            norm_q.popleft()()
        # remaining deferred output projections, then the final block's, on
        # rotating PSUM tags so the matmuls don't serialize on a single bank
        while oproj_g:
            oproj_g.popleft()()
        tags = [("po", opool), ("ps", ppool), ("pv0", vpool), ("pv1", vpool)]
        for m in range(D // P):
            ptag, pool = tags[m % 4]
            oproj_mtile(NQB - 1, m, ptag=ptag, pool=pool, gp=(m % 2 == 1))()
        if dbg is not None:
            nc.sync.dma_start(dbg["qT"], qT_sb)
            nc.sync.dma_start(dbg["kT"], kT_sb)
            nc.sync.dma_start(dbg["v"], v_sb)
            nc.sync.dma_start(dbg["attnT"], attnT_sb)


def build(debug_out=False, dbg_block=3):
    nc = bacc.Bacc(
        "TRN2",
        target_bir_lowering=False,
        debug=False,
        enable_asserts=False,
    )
    xT = nc.dram_tensor("xT", [D, S], BF16, kind="ExternalInput").ap()
    wqT = nc.dram_tensor("wqT", [D, P], BF16, kind="ExternalInput").ap()
    wkT = nc.dram_tensor("wkT", [D, P], BF16, kind="ExternalInput").ap()
    wvT = nc.dram_tensor("wvT", [D, P], BF16, kind="ExternalInput").ap()
    woT = nc.dram_tensor("woT", [P, D], BF16, kind="ExternalInput").ap()
    bqk = nc.dram_tensor("bqk", [3, P], F32, kind="ExternalInput").ap()
    masks = nc.dram_tensor("masks", [P, 4, NQ], BF16, kind="ExternalInput").ap()
    outT = nc.dram_tensor("outT", [D, S], F16, kind="ExternalOutput").ap()
    dbg = None
    if debug_out:
        dbg = {
            "qT": nc.dram_tensor("dbg_qT", [P, S], BF16, kind="ExternalOutput").ap(),
            "kT": nc.dram_tensor("dbg_kT", [P, S], BF16, kind="ExternalOutput").ap(),
            "v": nc.dram_tensor("dbg_v", [P, S // P, 130], BF16, kind="ExternalOutput").ap(),
            "attnT": nc.dram_tensor("dbg_attnT", [P, S], BF16, kind="ExternalOutput").ap(),
            "block": dbg_block,
        }

    with tile.TileContext(nc) as tc:
        _emit(tc, xT, wqT, wkT, wvT, woT, bqk, masks, outT, dbg=dbg)
    nc.compile()
    return nc


def _make_masks():
    k = np.arange(P)[:, None]
    q = np.arange(NQ)[None, :]
    m = np.zeros((P, 4, NQ), np.float32)
    for j in range(4):
        m[:, j, :] = ((KT * j + k) <= q).astype(np.float32)
    return m.astype(ml_dtypes.bfloat16)


_STATE = {}


def _prep_inputs(x, Wq, bq, Wk, bk, Wv, bv, Wo, bo):
    bf = ml_dtypes.bfloat16
    xT = np.ascontiguousarray(np.asarray(x, np.float32).reshape(S, D).T).astype(bf)
    masks = _make_masks()
    Wq = np.asarray(Wq, np.float32)
    Wk = np.asarray(Wk, np.float32)
    Wv = np.asarray(Wv, np.float32)
    Wo = np.asarray(Wo, np.float32)
    bq = np.asarray(bq, np.float32)
    bk = np.asarray(bk, np.float32)
    bv = np.asarray(bv, np.float32)
    in_maps = []
    for c in range(N_CORES):
        r = slice(c * P, (c + 1) * P)
        in_maps.append({
            "xT": xT,
            "wqT": np.ascontiguousarray(Wq[r].T).astype(bf),
            "wkT": np.ascontiguousarray(Wk[r].T).astype(bf),
            "wvT": np.ascontiguousarray(Wv[r].T).astype(bf),
            "woT": np.ascontiguousarray(Wo[:, r].T).astype(bf),
            "bqk": np.stack([bq[r], bk[r], bv[r]]),
            "masks": masks,
        })
    return in_maps


def kernel(x, Wq, bq, Wk, bk, Wv, bv, Wo, bo):
    if "nc" not in _STATE:
        _STATE["nc"] = build()
    nc = _STATE["nc"]
    in_maps = _prep_inputs(x, Wq, bq, Wk, bk, Wv, bv, Wo, bo)
    res = run_bass_kernel_spmd(nc, in_maps, core_ids=list(range(N_CORES)))
    total = res.results[0]["outT"].astype(np.float32, copy=True)
    for c in range(1, N_CORES):
        total += res.results[c]["outT"].astype(np.float32)
    out = total.T + np.asarray(bo, np.float32)[None, :]
    return np.ascontiguousarray(out, dtype=np.float32).reshape(1, S, D)


# revision 33
# speedup vs baseline: 1.0322x; 1.0322x over previous
"""Multi-head causal self-attention (B=1, S=4096, D=1024, H=16) on 8 TRN2 cores.

Sharding: 2 heads per core (head/tensor parallel). Each core computes its
heads' Q/K/V projections, causal flash attention, and a partial output
projection against its 128 columns of Wo. The host sums the 8 partials and
adds the output bias.

Device layouts (per core, bf16 compute):
  - x is fed transposed:  xT [D=1024, S=4096]   (model dim on partitions)
  - Q^T, K^T [128, 4096]: per-core head dims on partitions (h0: 0-63, h1: 64-127)
  - V natural [4096, 130]: per seq-tile [128, 65*2] = [V_h0 | ones | V_h1 | ones]
    The ones column makes the PV matmul also produce the softmax denominator.
  - scores are computed transposed S^T[k, q] so the PV matmul needs no
    transposition; softmax is exp-only (scores are bounded, no max-subtract).
  - the per-query 1/denominator is broadcast across the 64 head-feature
    partitions with K=1 ones-vector PE matmuls (h1 at tile-position column
    64), then per-head multiplies write normalized attnT rows directly.
  - output is written transposed outT [1024, 4096] fp16 (partial; host sums).
"""

import numpy as np
import ml_dtypes
from contextlib import ExitStack

import concourse.bass as bass
import concourse.tile as tile
from concourse import bacc, mybir
from concourse.bass_utils import run_bass_kernel_spmd

P = 128
S = 4096
D = 1024
DH = 64
N_CORES = 8
SCALE = 1.0 / 8.0  # 1/sqrt(64)
NQ = 512           # query block (matmul free dim)
KT = 128           # key tile (contraction partitions)
NQB = S // NQ      # 8 query blocks
NKT = S // KT      # 32 key tiles
KO = D // P        # 8 contraction subtiles over the model dim

BF16 = mybir.dt.bfloat16
F16 = mybir.dt.float16
F32 = mybir.dt.float32
EXP = mybir.ActivationFunctionType.Exp
ADD = mybir.AluOpType.add


def _emit(tc, xT, wqT, wkT, wvT, woT, bqk, masks, outT, dbg=None):
    nc = tc.nc
    with ExitStack() as ctx:
        from collections import deque
        from concourse.masks import make_identity

        const = ctx.enter_context(tc.tile_pool(name="const", bufs=1))

        # DMA order follows first-use: wq + x chunk-0-lo unblock the first
        # projection matmuls; don't queue anything behind the 8MB of xT.
        ones64 = const.tile([1, DH], BF16)
        nc.vector.memset(ones64, 1.0)

        xT_r = xT.rearrange("(ko p) n -> p ko n", p=P)
        xT_sb = const.tile([P, KO, S], BF16)
        wq_sb = const.tile([P, KO, P], BF16)
        nc.sync.dma_start(wq_sb, wqT.rearrange("(ko p) m -> p ko m", p=P))
        nc.sync.dma_start(xT_sb[:, 0:4, 0:NQ], xT_r[:, 0:4, 0:NQ])
        wk_sb = const.tile([P, KO, P], BF16)
        nc.sync.dma_start(wk_sb, wkT.rearrange("(ko p) m -> p ko m", p=P))
        nc.sync.dma_start(xT_sb[:, 4:8, 0:NQ], xT_r[:, 4:8, 0:NQ])
        wv_sb = const.tile([P, KO, P], BF16)
        nc.sync.dma_start(wv_sb, wvT.rearrange("(ko p) m -> p ko m", p=P))
        bqk_sb = const.tile([P, 3], F32)
        nc.sync.dma_start(bqk_sb, bqk.rearrange("b p -> p b"))
        masks_sb = const.tile([P, 4, NQ], BF16)
        nc.sync.dma_start(masks_sb, masks)
        nc.sync.dma_start(xT_sb[:, :, NQ:2 * NQ], xT_r[:, :, NQ:2 * NQ])
        wo_sb = const.tile([P, D], BF16)
        nc.sync.dma_start(wo_sb, woT)
        for n in range(2, NQB):  # chunked so projections can start early
            nsl = slice(n * NQ, (n + 1) * NQ)
            nc.sync.dma_start(xT_sb[:, :, nsl], xT_r[:, :, nsl])

        qT_sb = const.tile([P, S], BF16)
        kT_sb = const.tile([P, S], BF16)
        vT_sb = const.tile([P, S], BF16)
        v_sb = const.tile([P, S // P, 130], BF16)
        attnT_sb = const.tile([P, S], BF16)
        nc.gpsimd.memset(v_sb, 1.0)  # presets the two ones-columns

        ident = const.tile([P, P], BF16)
        make_identity(nc, ident)

        # Warm the PE clock (HAM) with throwaway matmuls while the input DMAs
        # land: ~20 matmuls of N=128 ~ 3us of sustained PE activity.
        with tc.tile_pool(name="warm_psum", bufs=1, space="PSUM") as wpool:
            wt = wpool.tile([P, P], F32)
            for _ in range(32):
                nc.tensor.matmul(wt, lhsT=ident, rhs=ident, start=True, stop=True)

        # PSUM budget (8 banks): spool 4 (two [128,2,512] score slabs),
        # vpool 2 (pv0/pv1 accumulators), ppool 1 (proj accum / V transpose),
        # opool 1 (output projection / reciprocal broadcast).
        spool = ctx.enter_context(tc.tile_pool(name="score_psum", bufs=2, space="PSUM"))
        vpool = ctx.enter_context(tc.tile_pool(name="pv_psum", bufs=1, space="PSUM"))
        ppool = ctx.enter_context(tc.tile_pool(name="proj_psum", bufs=1, space="PSUM"))
        opool = ctx.enter_context(tc.tile_pool(name="oproj_psum", bufs=1, space="PSUM"))
        work = ctx.enter_context(tc.tile_pool(name="work", bufs=5))
        nwork = ctx.enter_context(tc.tile_pool(name="nwork", bufs=3))

        def proj_chunk(bcol, w_sb, dst, n):
            """Two pacing items of 4 accumulation matmuls each (shared psum)."""
            state = {}

            def emit_lo():
                ps = ppool.tile([P, NQ], F32, tag="ps", name=f"ps_{bcol}_{n}")
                state["ps"] = ps
                for kt in range(KO // 2):
                    nc.tensor.matmul(
                        ps,
                        lhsT=w_sb[:, kt, :],
                        rhs=xT_sb[:, kt, n * NQ:(n + 1) * NQ],
                        start=(kt == 0),
                        stop=False,
                    )

            def emit_hi():
                ps = state["ps"]
                for kt in range(KO // 2, KO):
                    nc.tensor.matmul(
                        ps,
                        lhsT=w_sb[:, kt, :],
                        rhs=xT_sb[:, kt, n * NQ:(n + 1) * NQ],
                        start=False,
                        stop=(kt == KO - 1),
                    )
                nc.vector.tensor_tensor(
                    dst[:, n * NQ:(n + 1) * NQ],
                    ps,
                    bqk_sb[:, bcol:bcol + 1].to_broadcast([P, NQ]),
                    op=ADD,
                )

            return [emit_lo, emit_hi]

        def v_transpose(t):
            def emit():
                tp = ppool.tile([P, P], BF16, tag="ps", name=f"tp_{t}")
                nc.tensor.transpose(tp, vT_sb[:, t * P:(t + 1) * P], ident)
                nc.vector.tensor_copy(
                    v_sb[:, t, :].rearrange("p (h x) -> p h x", x=65)[:, :, 0:DH],
                    tp.rearrange("p (h x) -> p h x", x=DH),
                )
            return emit

        def proj_ops(nb):
            ops = []
            ops += proj_chunk(0, wq_sb, qT_sb, nb)
            ops += proj_chunk(1, wk_sb, kT_sb, nb)
            ops += proj_chunk(2, wv_sb, vT_sb, nb)
            ops += [v_transpose(t) for t in range(4 * nb, 4 * nb + 4)]
            return ops

        def oproj_mtile(b, m, ptag="po", pool=None, gp=False, cs=None):
            def emit():
                c0, c1 = (cs.start, cs.stop) if cs else (0, NQ)
                n = c1 - c0
                qsl = slice(b * NQ + c0, b * NQ + c1)
                po = (pool or opool).tile(
                    [P, n], F32, tag=ptag, name=f"po_{b}_{m}_{c0}")
                nc.tensor.matmul(
                    po,
                    lhsT=wo_sb[:, m * P:(m + 1) * P],
                    rhs=attnT_sb[:, qsl],
                    start=True,
                    stop=True,
                )
                ot = work.tile([P, n], F16, tag="ot", name=f"ot_{b}_{m}_{c0}")
                if gp:
                    nc.scalar.copy(ot, po)
                else:
                    nc.vector.tensor_copy(ot, po)
                nc.sync.dma_start(
                    outT.rearrange("(mo p) n -> p mo n", p=P)[:, m, qsl], ot
                )
            return emit

        # block 0's projections up front
        for op in proj_ops(0):
            op()

        # Deferred output-projection queue: blocks 0-6 enqueue their 8
        # m-tiles after their attnT is normalized, but the work is only
        # popped during blocks >= 5, whose long kt loops are exp-bound -
        # the PE needs ready filler there to avoid head-of-line stalls on
        # PV-waiting-for-exp (each stall also drops the PE p-state).
        oproj_g = deque()

        def normalize_part1(b, pvs, cs=None, sfx=""):
            """End-of-block: drain PV psum to SBUF + reciprocal of the
            denominators. Frees the pv banks as early as possible. h1's
            rows are staged at partition base 64 (cross-partition DVE copy)
            so its normalize multiply can write attnT rows 64-127 directly."""
            cs = cs or slice(0, NQ)
            n = cs.stop - cs.start
            dcp = nwork.tile([1, 2, n], F32, tag="dcp", name=f"dcp_{b}{sfx}")
            pvS = nwork.tile([P, n], F32, tag="pvS", name=f"pvS_{b}{sfx}")
            for h in (0, 1):
                # denominator rows -> one base-0 tile (heads packed in the
                # free dim: custom-DVE recip mishandles nonzero partition
                # bases, and partition-base-1 writes are illegal)
                nc.vector.tensor_copy(dcp[:, h, :], pvs[h][DH:DH + 1, cs])
                nc.vector.tensor_copy(
                    pvS[h * DH:(h + 1) * DH, :], pvs[h][0:DH, cs])
            nc.vector.reciprocal_approx_fast(dcp, dcp)
            dcpb = nwork.tile([1, 2, n], BF16, tag="dcpb", name=f"dcpb_{b}{sfx}")
            nc.vector.tensor_copy(dcpb, dcp)
            return pvS, dcpb

        def normalize_part2(b, pvS, dcpb, cs=None, sfx=""):
            """Broadcast 1/denom over the head-feature partitions with K=1
            ones-vector matmuls (into the pv banks just freed by part1,
            h1's at tile-position column 64), then two multiplies write
            normalized attnT rows directly. Deferred into the next block's
            kt loop so the block boundary never stalls the PE queue."""
            def emit():
                cs_ = cs or slice(0, NQ)
                n = cs_.stop - cs_.start
                qsl = slice(b * NQ + cs_.start, b * NQ + cs_.stop)
                bbp = [
                    vpool.tile([P, n], F32, tag=f"pv{h}", name=f"bbp{h}_{b}{sfx}")
                    for h in (0, 1)
                ]
                for h in (0, 1):
                    nc.tensor.matmul(
                        bbp[h][h * DH:(h + 1) * DH, :],
                        lhsT=ones64,
                        rhs=dcpb[:, h, :],
                        start=True,
                        stop=True,
                    )
                    nc.vector.tensor_mul(
                        attnT_sb[h * DH:(h + 1) * DH, qsl],
                        pvS[h * DH:(h + 1) * DH, :],
                        bbp[h][h * DH:(h + 1) * DH, :],
                    )
                if b < NQB - 1:
                    oproj_g.extend(oproj_mtile(b, m) for m in range(D // P))
            return emit

        def pe_warm(n):
            # throwaway matmuls: keep the PE activity monitor dense so the
            # clock does not downshift while waiting on the exp stream
            dpo = opool.tile([P, P], F32, tag="po", name="dpo")
            for _ in range(n):
                nc.tensor.matmul(dpo, lhsT=ident, rhs=ident, start=True, stop=True)

        norm_q = deque()
        for b in range(NQB):
            nk = 4 * (b + 1)  # causal: only key tiles up to the diagonal
            proj_q = deque(proj_ops(b + 1)) if b + 1 < NQB else deque()
            pvs = [
                vpool.tile([DH + 1, NQ], F32, tag=f"pv{h}", name=f"pv{h}_{b}")
                for h in (0, 1)
            ]

            def emit_pv(st):
                pT, kt_, q0, nq, ki_ = st
                for h in (0, 1):
                    nc.tensor.matmul(
                        pvs[h][:, q0:],
                        lhsT=v_sb[:, kt_, h * 65:(h + 1) * 65],
                        rhs=pT[:, h, :nq],
                        start=(ki_ == 0),
                        stop=(ki_ == nk - 1),
                    )

            prev = None  # PV runs one k-tile behind the scores/exp pipeline
            for ki, kt in enumerate(range(nk)):
                j = kt - 4 * b  # >= 0 on causal-diagonal key tiles
                # on diagonal tiles only queries >= 128j can attend this tile
                q0 = max(0, j) * KT
                nq = NQ - q0
                qs0 = b * NQ + q0
                slab = spool.tile([P, 2, NQ], F32, tag="slab")
                for h in (0, 1):
                    nc.tensor.matmul(
                        slab[:, h, :nq],
                        lhsT=kT_sb[h * DH:(h + 1) * DH, kt * KT:(kt + 1) * KT],
                        rhs=qT_sb[h * DH:(h + 1) * DH, qs0:qs0 + nq],
                        start=True,
                        stop=True,
                    )
                pT = work.tile([P, 2, NQ], BF16, tag="pT")
                nc.scalar.activation(pT[:, :, :nq], slab[:, :, :nq], EXP, scale=SCALE)
                if j >= 0:
                    for h in (0, 1):
                        nc.vector.tensor_mul(
                            pT[:, h, :nq],
                            pT[:, h, :nq],
                            masks_sb[:, j, q0:],
                        )
                if prev is not None:
                    emit_pv(prev)
                prev = (pT, kt, q0, nq, ki)
                # PE filler priority: finish the previous block's normalize
                # (delayed 2 kt so its vector chain clears first), then next
                # block's projections, then deferred output projections
                # (concentrated in the exp-bound blocks 6-7), else throwaway
                # warm matmuls so the PE clock stays up while exp catches up
                if norm_q and ki >= 3:
                    norm_q.popleft()()
                elif proj_q:
                    proj_q.popleft()()
                elif oproj_g and b == 6 and ki % 2 == 0:
                    oproj_g.popleft()()
                elif oproj_g and b == 7:
                    oproj_g.popleft()()
                    if ki >= 17 and oproj_g:
                        oproj_g.popleft()()
                elif b >= 4:
                    pe_warm(3)
            emit_pv(prev)
            while norm_q:
                norm_q.popleft()()
            while proj_q:
                proj_q.popleft()()
            pvS, dcpb = normalize_part1(b, pvs)
            norm_q.append(normalize_part2(b, pvS, dcpb))
        while norm_q:
            norm_q.popleft()()
        # remaining deferred output projections, then the final block's, on
        # rotating PSUM tags so the matmuls don't serialize on a single bank
        while oproj_g:
            oproj_g.popleft()()
        tags = [("po", opool), ("ps", ppool), ("pv0", vpool), ("pv1", vpool)]
        for m in range(D // P):
            ptag, pool = tags[m % 4]
            oproj_mtile(NQB - 1, m, ptag=ptag, pool=pool, gp=(m % 2 == 1))()
        if dbg is not None:
            nc.sync.dma_start(dbg["qT"], qT_sb)
            nc.sync.dma_start(dbg["kT"], kT_sb)
            nc.sync.dma_start(dbg["v"], v_sb)
            nc.sync.dma_start(dbg["attnT"], attnT_sb)


def build(debug_out=False, dbg_block=3):
    nc = bacc.Bacc(
        "TRN2",
        target_bir_lowering=False,
        debug=False,
        enable_asserts=False,
    )
    xT = nc.dram_tensor("xT", [D, S], BF16, kind="ExternalInput").ap()
    wqT = nc.dram_tensor("wqT", [D, P], BF16, kind="ExternalInput").ap()
    wkT = nc.dram_tensor("wkT", [D, P], BF16, kind="ExternalInput").ap()
    wvT = nc.dram_tensor("wvT", [D, P], BF16, kind="ExternalInput").ap()
    woT = nc.dram_tensor("woT", [P, D], BF16, kind="ExternalInput").ap()
    bqk = nc.dram_tensor("bqk", [3, P], F32, kind="ExternalInput").ap()
    masks = nc.dram_tensor("masks", [P, 4, NQ], BF16, kind="ExternalInput").ap()
    outT = nc.dram_tensor("outT", [D, S], F16, kind="ExternalOutput").ap()
    dbg = None
    if debug_out:
        dbg = {
            "qT": nc.dram_tensor("dbg_qT", [P, S], BF16, kind="ExternalOutput").ap(),
            "kT": nc.dram_tensor("dbg_kT", [P, S], BF16, kind="ExternalOutput").ap(),
            "v": nc.dram_tensor("dbg_v", [P, S // P, 130], BF16, kind="ExternalOutput").ap(),
            "attnT": nc.dram_tensor("dbg_attnT", [P, S], BF16, kind="ExternalOutput").ap(),
            "block": dbg_block,
        }

    with tile.TileContext(nc) as tc:
        _emit(tc, xT, wqT, wkT, wvT, woT, bqk, masks, outT, dbg=dbg)
    nc.compile()
    return nc


def _make_masks():
    k = np.arange(P)[:, None]
    q = np.arange(NQ)[None, :]
    m = np.zeros((P, 4, NQ), np.float32)
    for j in range(4):
        m[:, j, :] = ((KT * j + k) <= q).astype(np.float32)
    return m.astype(ml_dtypes.bfloat16)


_STATE = {}


def _prep_inputs(x, Wq, bq, Wk, bk, Wv, bv, Wo, bo):
    bf = ml_dtypes.bfloat16
    xT = np.ascontiguousarray(np.asarray(x, np.float32).reshape(S, D).T).astype(bf)
    masks = _make_masks()
    Wq = np.asarray(Wq, np.float32)
    Wk = np.asarray(Wk, np.float32)
    Wv = np.asarray(Wv, np.float32)
    Wo = np.asarray(Wo, np.float32)
    bq = np.asarray(bq, np.float32)
    bk = np.asarray(bk, np.float32)
    bv = np.asarray(bv, np.float32)
    in_maps = []
    for c in range(N_CORES):
        r = slice(c * P, (c + 1) * P)
        in_maps.append({
            "xT": xT,
            "wqT": np.ascontiguousarray(Wq[r].T).astype(bf),
            "wkT": np.ascontiguousarray(Wk[r].T).astype(bf),
            "wvT": np.ascontiguousarray(Wv[r].T).astype(bf),
            "woT": np.ascontiguousarray(Wo[:, r].T).astype(bf),
            "bqk": np.stack([bq[r], bk[r], bv[r]]),
            "masks": masks,
        })
    return in_maps


def kernel(x, Wq, bq, Wk, bk, Wv, bv, Wo, bo):
    if "nc" not in _STATE:
        _STATE["nc"] = build()
    nc = _STATE["nc"]
    in_maps = _prep_inputs(x, Wq, bq, Wk, bk, Wv, bv, Wo, bo)
    res = run_bass_kernel_spmd(nc, in_maps, core_ids=list(range(N_CORES)))
    total = res.results[0]["outT"].astype(np.float32, copy=True)
    for c in range(1, N_CORES):
        total += res.results[c]["outT"].astype(np.float32)
    out = total.T + np.asarray(bo, np.float32)[None, :]
    return np.ascontiguousarray(out, dtype=np.float32).reshape(1, S, D)


# revision 34
# speedup vs baseline: 1.0343x; 1.0020x over previous
"""Multi-head causal self-attention (B=1, S=4096, D=1024, H=16) on 8 TRN2 cores.

Sharding: 2 heads per core (head/tensor parallel). Each core computes its
heads' Q/K/V projections, causal flash attention, and a partial output
projection against its 128 columns of Wo. The host sums the 8 partials and
adds the output bias.

Device layouts (per core, bf16 compute):
  - x is fed transposed:  xT [D=1024, S=4096]   (model dim on partitions)
  - Q^T, K^T [128, 4096]: per-core head dims on partitions (h0: 0-63, h1: 64-127)
  - V natural [4096, 130]: per seq-tile [128, 65*2] = [V_h0 | ones | V_h1 | ones]
    The ones column makes the PV matmul also produce the softmax denominator.
  - scores are computed transposed S^T[k, q] so the PV matmul needs no
    transposition; softmax is exp-only (scores are bounded, no max-subtract).
  - the per-query 1/denominator is broadcast across the 64 head-feature
    partitions with K=1 ones-vector PE matmuls (h1 at tile-position column
    64), then per-head multiplies write normalized attnT rows directly.
  - output is written transposed outT [1024, 4096] fp16 (partial; host sums).
"""

import numpy as np
import ml_dtypes
from contextlib import ExitStack

import concourse.bass as bass
import concourse.tile as tile
from concourse import bacc, mybir
from concourse.bass_utils import run_bass_kernel_spmd

P = 128
S = 4096
D = 1024
DH = 64
N_CORES = 8
SCALE = 1.0 / 8.0  # 1/sqrt(64)
NQ = 512           # query block (matmul free dim)
KT = 128           # key tile (contraction partitions)
NQB = S // NQ      # 8 query blocks
NKT = S // KT      # 32 key tiles
KO = D // P        # 8 contraction subtiles over the model dim

BF16 = mybir.dt.bfloat16
F16 = mybir.dt.float16
F32 = mybir.dt.float32
EXP = mybir.ActivationFunctionType.Exp
ADD = mybir.AluOpType.add


def _emit(tc, xT, wqT, wkT, wvT, woT, bqk, masks, outT, dbg=None):
    nc = tc.nc
    with ExitStack() as ctx:
        from collections import deque
        from concourse.masks import make_identity

        const = ctx.enter_context(tc.tile_pool(name="const", bufs=1))

        # DMA order follows first-use: wq + x chunk-0-lo unblock the first
        # projection matmuls; don't queue anything behind the 8MB of xT.
        ones64 = const.tile([1, DH], BF16)
        nc.vector.memset(ones64, 1.0)

        xT_r = xT.rearrange("(ko p) n -> p ko n", p=P)
        xT_sb = const.tile([P, KO, S], BF16)
        wq_sb = const.tile([P, KO, P], BF16)
        nc.sync.dma_start(wq_sb, wqT.rearrange("(ko p) m -> p ko m", p=P))
        nc.sync.dma_start(xT_sb[:, 0:4, 0:NQ], xT_r[:, 0:4, 0:NQ])
        wk_sb = const.tile([P, KO, P], BF16)
        nc.sync.dma_start(wk_sb, wkT.rearrange("(ko p) m -> p ko m", p=P))
        nc.sync.dma_start(xT_sb[:, 4:8, 0:NQ], xT_r[:, 4:8, 0:NQ])
        wv_sb = const.tile([P, KO, P], BF16)
        nc.sync.dma_start(wv_sb, wvT.rearrange("(ko p) m -> p ko m", p=P))
        bqk_sb = const.tile([P, 3], F32)
        nc.sync.dma_start(bqk_sb, bqk.rearrange("b p -> p b"))
        masks_sb = const.tile([P, 4, NQ], BF16)
        nc.sync.dma_start(masks_sb, masks)
        nc.sync.dma_start(xT_sb[:, :, NQ:2 * NQ], xT_r[:, :, NQ:2 * NQ])
        wo_sb = const.tile([P, D], BF16)
        nc.sync.dma_start(wo_sb, woT)
        for n in range(2, NQB):  # chunked so projections can start early
            nsl = slice(n * NQ, (n + 1) * NQ)
            nc.sync.dma_start(xT_sb[:, :, nsl], xT_r[:, :, nsl])

        qT_sb = const.tile([P, S], BF16)
        kT_sb = const.tile([P, S], BF16)
        vT_sb = const.tile([P, S], BF16)
        v_sb = const.tile([P, S // P, 130], BF16)
        attnT_sb = const.tile([P, S], BF16)
        nc.gpsimd.memset(v_sb, 1.0)  # presets the two ones-columns

        ident = const.tile([P, P], BF16)
        make_identity(nc, ident)

        # Warm the PE clock (HAM) with throwaway matmuls while the input DMAs
        # land: ~20 matmuls of N=128 ~ 3us of sustained PE activity.
        with tc.tile_pool(name="warm_psum", bufs=1, space="PSUM") as wpool:
            wt = wpool.tile([P, P], F32)
            for _ in range(32):
                nc.tensor.matmul(wt, lhsT=ident, rhs=ident, start=True, stop=True)

        # PSUM budget (8 banks): spool 4 (two [128,2,512] score slabs),
        # vpool 2 (pv0/pv1 accumulators), ppool 1 (proj accum / V transpose),
        # opool 1 (output projection / reciprocal broadcast).
        spool = ctx.enter_context(tc.tile_pool(name="score_psum", bufs=2, space="PSUM"))
        vpool = ctx.enter_context(tc.tile_pool(name="pv_psum", bufs=1, space="PSUM"))
        ppool = ctx.enter_context(tc.tile_pool(name="proj_psum", bufs=1, space="PSUM"))
        opool = ctx.enter_context(tc.tile_pool(name="oproj_psum", bufs=1, space="PSUM"))
        work = ctx.enter_context(tc.tile_pool(name="work", bufs=5))
        nwork = ctx.enter_context(tc.tile_pool(name="nwork", bufs=3))

        def proj_chunk(bcol, w_sb, dst, n):
            """Two pacing items of 4 accumulation matmuls each (shared psum)."""
            state = {}

            def emit_lo():
                ps = ppool.tile([P, NQ], F32, tag="ps", name=f"ps_{bcol}_{n}")
                state["ps"] = ps
                for kt in range(KO // 2):
                    nc.tensor.matmul(
                        ps,
                        lhsT=w_sb[:, kt, :],
                        rhs=xT_sb[:, kt, n * NQ:(n + 1) * NQ],
                        start=(kt == 0),
                        stop=False,
                    )

            def emit_hi():
                ps = state["ps"]
                for kt in range(KO // 2, KO):
                    nc.tensor.matmul(
                        ps,
                        lhsT=w_sb[:, kt, :],
                        rhs=xT_sb[:, kt, n * NQ:(n + 1) * NQ],
                        start=False,
                        stop=(kt == KO - 1),
                    )
                nc.vector.tensor_tensor(
                    dst[:, n * NQ:(n + 1) * NQ],
                    ps,
                    bqk_sb[:, bcol:bcol + 1].to_broadcast([P, NQ]),
                    op=ADD,
                )

            return [emit_lo, emit_hi]

        def v_transpose(t):
            def emit():
                tp = ppool.tile([P, P], BF16, tag="ps", name=f"tp_{t}")
                nc.tensor.transpose(tp, vT_sb[:, t * P:(t + 1) * P], ident)
                nc.vector.tensor_copy(
                    v_sb[:, t, :].rearrange("p (h x) -> p h x", x=65)[:, :, 0:DH],
                    tp.rearrange("p (h x) -> p h x", x=DH),
                )
            return emit

        def proj_ops(nb):
            ops = []
            ops += proj_chunk(0, wq_sb, qT_sb, nb)
            ops += proj_chunk(1, wk_sb, kT_sb, nb)
            ops += proj_chunk(2, wv_sb, vT_sb, nb)
            ops += [v_transpose(t) for t in range(4 * nb, 4 * nb + 4)]
            return ops

        def oproj_mtile(b, m, ptag="po", pool=None, gp=False, cs=None):
            def emit():
                c0, c1 = (cs.start, cs.stop) if cs else (0, NQ)
                n = c1 - c0
                qsl = slice(b * NQ + c0, b * NQ + c1)
                po = (pool or opool).tile(
                    [P, n], F32, tag=ptag, name=f"po_{b}_{m}_{c0}")
                nc.tensor.matmul(
                    po,
                    lhsT=wo_sb[:, m * P:(m + 1) * P],
                    rhs=attnT_sb[:, qsl],
                    start=True,
                    stop=True,
                )
                ot = work.tile([P, n], F16, tag="ot", name=f"ot_{b}_{m}_{c0}")
                if gp:
                    nc.scalar.copy(ot, po)
                else:
                    nc.vector.tensor_copy(ot, po)
                nc.sync.dma_start(
                    outT.rearrange("(mo p) n -> p mo n", p=P)[:, m, qsl], ot
                )
            return emit

        # block 0's projections up front
        for op in proj_ops(0):
            op()

        # Deferred output-projection queue: blocks 0-6 enqueue their 8
        # m-tiles after their attnT is normalized, but the work is only
        # popped during blocks >= 5, whose long kt loops are exp-bound -
        # the PE needs ready filler there to avoid head-of-line stalls on
        # PV-waiting-for-exp (each stall also drops the PE p-state).
        oproj_g = deque()

        def normalize_part1(b, pvs, cs=None, sfx=""):
            """End-of-block: drain PV psum to SBUF + reciprocal of the
            denominators. Frees the pv banks as early as possible. h1's
            rows are staged at partition base 64 (cross-partition DVE copy)
            so its normalize multiply can write attnT rows 64-127 directly."""
            cs = cs or slice(0, NQ)
            n = cs.stop - cs.start
            dcp = nwork.tile([1, 2, n], F32, tag="dcp", name=f"dcp_{b}{sfx}")
            pvS = nwork.tile([P, n], F32, tag="pvS", name=f"pvS_{b}{sfx}")
            for h in (0, 1):
                # denominator rows -> one base-0 tile (heads packed in the
                # free dim: custom-DVE recip mishandles nonzero partition
                # bases, and partition-base-1 writes are illegal)
                nc.vector.tensor_copy(dcp[:, h, :], pvs[h][DH:DH + 1, cs])
                nc.vector.tensor_copy(
                    pvS[h * DH:(h + 1) * DH, :], pvs[h][0:DH, cs])
            nc.vector.reciprocal_approx_fast(dcp, dcp)
            dcpb = nwork.tile([1, 2, n], BF16, tag="dcpb", name=f"dcpb_{b}{sfx}")
            nc.vector.tensor_copy(dcpb, dcp)
            return pvS, dcpb

        def normalize_part2(b, pvS, dcpb, cs=None, sfx=""):
            """Broadcast 1/denom over the head-feature partitions with K=1
            ones-vector matmuls (into the pv banks just freed by part1,
            h1's at tile-position column 64), then two multiplies write
            normalized attnT rows directly. Deferred into the next block's
            kt loop so the block boundary never stalls the PE queue."""
            def emit():
                cs_ = cs or slice(0, NQ)
                n = cs_.stop - cs_.start
                qsl = slice(b * NQ + cs_.start, b * NQ + cs_.stop)
                bbp = [
                    vpool.tile([P, n], F32, tag=f"pv{h}", name=f"bbp{h}_{b}{sfx}")
                    for h in (0, 1)
                ]
                for h in (0, 1):
                    nc.tensor.matmul(
                        bbp[h][h * DH:(h + 1) * DH, :],
                        lhsT=ones64,
                        rhs=dcpb[:, h, :],
                        start=True,
                        stop=True,
                    )
                    nc.vector.tensor_mul(
                        attnT_sb[h * DH:(h + 1) * DH, qsl],
                        pvS[h * DH:(h + 1) * DH, :],
                        bbp[h][h * DH:(h + 1) * DH, :],
                    )
                if b < NQB - 1:
                    oproj_g.extend(oproj_mtile(b, m) for m in range(D // P))
            return emit

        def pe_warm(n):
            # throwaway matmuls: keep the PE activity monitor dense so the
            # clock does not downshift while waiting on the exp stream
            dpo = opool.tile([P, P], F32, tag="po", name="dpo")
            for _ in range(n):
                nc.tensor.matmul(dpo, lhsT=ident, rhs=ident, start=True, stop=True)

        norm_q = deque()
        for b in range(NQB):
            nk = 4 * (b + 1)  # causal: only key tiles up to the diagonal
            proj_q = deque(proj_ops(b + 1)) if b + 1 < NQB else deque()
            pvs = [
                vpool.tile([DH + 1, NQ], F32, tag=f"pv{h}", name=f"pv{h}_{b}")
                for h in (0, 1)
            ]

            def emit_pv(st):
                pT, kt_, q0, nq, ki_ = st
                for h in (0, 1):
                    nc.tensor.matmul(
                        pvs[h][:, q0:],
                        lhsT=v_sb[:, kt_, h * 65:(h + 1) * 65],
                        rhs=pT[:, h, :nq],
                        start=(ki_ == 0),
                        stop=(ki_ == nk - 1),
                    )

            prev = None  # PV runs one k-tile behind the scores/exp pipeline
            for ki, kt in enumerate(range(nk)):
                j = kt - 4 * b  # >= 0 on causal-diagonal key tiles
                # on diagonal tiles only queries >= 128j can attend this tile
                q0 = max(0, j) * KT
                nq = NQ - q0
                qs0 = b * NQ + q0
                slab = spool.tile([P, 2, NQ], F32, tag="slab")
                for h in (0, 1):
                    nc.tensor.matmul(
                        slab[:, h, :nq],
                        lhsT=kT_sb[h * DH:(h + 1) * DH, kt * KT:(kt + 1) * KT],
                        rhs=qT_sb[h * DH:(h + 1) * DH, qs0:qs0 + nq],
                        start=True,
                        stop=True,
                    )
                pT = work.tile([P, 2, NQ], BF16, tag="pT")
                nc.scalar.activation(pT[:, :, :nq], slab[:, :, :nq], EXP, scale=SCALE)
                if j >= 0:
                    for h in (0, 1):
                        nc.vector.tensor_mul(
                            pT[:, h, :nq],
                            pT[:, h, :nq],
                            masks_sb[:, j, q0:],
                        )
                if prev is not None:
                    emit_pv(prev)
                prev = (pT, kt, q0, nq, ki)
                # PE filler priority: finish the previous block's normalize
                # (delayed 2 kt so its vector chain clears first), then next
                # block's projections, then deferred output projections
                # (concentrated in the exp-bound blocks 6-7), else throwaway
                # warm matmuls so the PE clock stays up while exp catches up
                if norm_q and ki >= 3:
                    norm_q.popleft()()
                elif proj_q:
                    proj_q.popleft()()
                elif oproj_g and b == 6:
                    oproj_g.popleft()()
                elif oproj_g and b == 7:
                    oproj_g.popleft()()
                    if ki >= 17 and oproj_g:
                        oproj_g.popleft()()
                elif b >= 4:
                    pe_warm(3)
            emit_pv(prev)
            while norm_q:
                norm_q.popleft()()
            while proj_q:
                proj_q.popleft()()
            pvS, dcpb = normalize_part1(b, pvs)
            norm_q.append(normalize_part2(b, pvS, dcpb))
        while norm_q:
            norm_q.popleft()()
        # remaining deferred output projections, then the final block's, on
        # rotating PSUM tags so the matmuls don't serialize on a single bank
        while oproj_g:
            oproj_g.popleft()()
        tags = [("po", opool), ("ps", ppool), ("pv0", vpool), ("pv1", vpool)]
        for m in range(D // P):
            ptag, pool = tags[m % 4]
            oproj_mtile(NQB - 1, m, ptag=ptag, pool=pool, gp=(m % 2 == 1))()
        if dbg is not None:
            nc.sync.dma_start(dbg["qT"], qT_sb)
            nc.sync.dma_start(dbg["kT"], kT_sb)
            nc.sync.dma_start(dbg["v"], v_sb)
            nc.sync.dma_start(dbg["attnT"], attnT_sb)


def build(debug_out=False, dbg_block=3):
    nc = bacc.Bacc(
        "TRN2",
        target_bir_lowering=False,
        debug=False,
        enable_asserts=False,
    )
    xT = nc.dram_tensor("xT", [D, S], BF16, kind="ExternalInput").ap()
    wqT = nc.dram_tensor("wqT", [D, P], BF16, kind="ExternalInput").ap()
    wkT = nc.dram_tensor("wkT", [D, P], BF16, kind="ExternalInput").ap()
    wvT = nc.dram_tensor("wvT", [D, P], BF16, kind="ExternalInput").ap()
    woT = nc.dram_tensor("woT", [P, D], BF16, kind="ExternalInput").ap()
    bqk = nc.dram_tensor("bqk", [3, P], F32, kind="ExternalInput").ap()
    masks = nc.dram_tensor("masks", [P, 4, NQ], BF16, kind="ExternalInput").ap()
    outT = nc.dram_tensor("outT", [D, S], F16, kind="ExternalOutput").ap()
    dbg = None
    if debug_out:
        dbg = {
            "qT": nc.dram_tensor("dbg_qT", [P, S], BF16, kind="ExternalOutput").ap(),
            "kT": nc.dram_tensor("dbg_kT", [P, S], BF16, kind="ExternalOutput").ap(),
            "v": nc.dram_tensor("dbg_v", [P, S // P, 130], BF16, kind="ExternalOutput").ap(),
            "attnT": nc.dram_tensor("dbg_attnT", [P, S], BF16, kind="ExternalOutput").ap(),
            "block": dbg_block,
        }

    with tile.TileContext(nc) as tc:
        _emit(tc, xT, wqT, wkT, wvT, woT, bqk, masks, outT, dbg=dbg)
    nc.compile()
    return nc


def _make_masks():
    k = np.arange(P)[:, None]
    q = np.arange(NQ)[None, :]
    m = np.zeros((P, 4, NQ), np.float32)
    for j in range(4):
        m[:, j, :] = ((KT * j + k) <= q).astype(np.float32)
    return m.astype(ml_dtypes.bfloat16)


_STATE = {}


def _prep_inputs(x, Wq, bq, Wk, bk, Wv, bv, Wo, bo):
    bf = ml_dtypes.bfloat16
    xT = np.ascontiguousarray(np.asarray(x, np.float32).reshape(S, D).T).astype(bf)
    masks = _make_masks()
    Wq = np.asarray(Wq, np.float32)
    Wk = np.asarray(Wk, np.float32)
    Wv = np.asarray(Wv, np.float32)
    Wo = np.asarray(Wo, np.float32)
    bq = np.asarray(bq, np.float32)
    bk = np.asarray(bk, np.float32)
    bv = np.asarray(bv, np.float32)
    in_maps = []
    for c in range(N_CORES):
        r = slice(c * P, (c + 1) * P)
        in_maps.append({
            "xT": xT,
            "wqT": np.ascontiguousarray(Wq[r].T).astype(bf),
            "wkT": np.ascontiguousarray(Wk[r].T).astype(bf),
            "wvT": np.ascontiguousarray(Wv[r].T).astype(bf),
            "woT": np.ascontiguousarray(Wo[:, r].T).astype(bf),
            "bqk": np.stack([bq[r], bk[r], bv[r]]),
            "masks": masks,
        })
    return in_maps


def kernel(x, Wq, bq, Wk, bk, Wv, bv, Wo, bo):
    if "nc" not in _STATE:
        _STATE["nc"] = build()
    nc = _STATE["nc"]
    in_maps = _prep_inputs(x, Wq, bq, Wk, bk, Wv, bv, Wo, bo)
    res = run_bass_kernel_spmd(nc, in_maps, core_ids=list(range(N_CORES)))
    total = res.results[0]["outT"].astype(np.float32, copy=True)
    for c in range(1, N_CORES):
        total += res.results[c]["outT"].astype(np.float32)
    out = total.T + np.asarray(bo, np.float32)[None, :]
    return np.ascontiguousarray(out, dtype=np.float32).reshape(1, S, D)
